# revision 1
# baseline (speedup 1.0000x reference)
"""Dechirp-STFT Trainium2 kernel.

Computes, for each of D=16 chirp hypotheses, a resampled (linear-interp)
version of each Hann-windowed signal frame followed by a 1024-point FFT.

Strategy
--------
Per chirp d the whole frame-wise operation (gather/lerp resample -> DFT) is a
single linear map on the 1024-sample frame, so we fold both into one dense
matrix M_d built on the host from `dlnf`:  X_d = frames @ M_d.
Only rFFT bins f=0..512 are computed on device (input frames are real, so
the upper half is the conjugate mirror, filled in on the host). Device rows
use the packed-rfft column order [re0, re1, im1, ..., re511, im511, re512]
(1024 cols; im0/im512 are identically zero and filled host-side), which makes
every matmul an exact 512-wide PSUM bank and every output row 4096B-aligned.

Sharding: D axis across the 8 NeuronCores (2 chirps per core). Every core
holds the full transposed frame matrix wT (1024 x 2048, frame count padded
2047->2048) and its two combined matrices (1024 x 1024 each). The device
kernel is a plain tiled matmul: out[c] = wT.T @ M_c, 128x512 output tiles,
K=1024 contraction (8 k-tiles), PSUM accumulation, DVE copyback, DMA out.

Dataflow details (from trace analysis):
- inputs are stored partition-major in DRAM and loaded as k-quarter pairs
  in lockstep across the Sync and Scalar HWDGE rings; a short warmup burst
  un-throttles the PE clock (HAM -> 2.4 GHz), then 4 partial-K accumulation
  groups chase the quarter arrivals so real matmuls overlap the input load;
- outputs are staged GRP=4 row-tiles at a time and written with 16KB
  per-partition descriptors on the Sync ring (partition-major DRAM layout,
  untangled on the host);
- dtype mode "fp16" (default) runs the PE at full 16-bit rate with 10
  mantissa bits of input precision (values here are O(1), well within range).
  Measured ~134.6us HW time, ~2.8e-4 relative error.
"""

import os
import numpy as np

K = 1024
HOP = 512
NW = 2047          # (1048576 - 1024) // 512 + 1
RWS = 2048         # padded row count (16 x 128 tiles)
NRT = RWS // 128   # 16 row tiles
GRP = 4            # row tiles per output staging group
NG = NRT // GRP
D = 16
NCORES = 8
DPC = D // NCORES  # chirps per core
NF = K // 2 + 1    # 513 rfft bins
# device computes 1024 cols per row in packed-rfft order:
# [re0, re1, im1, re2, im2, ..., re511, im511, re512]
# (im0 and im512 are identically zero and are filled on the host)
OCOLS = 1024
CT = 512           # matmul col-tile (2 x 512 = 1024)
NCT = OCOLS // CT
KT = K // 128      # 8 contraction tiles

# device dtype mode: "bf16" | "fp16" | "f32" | "f32r"
DEV_DT = os.environ.get("BASS_KERNEL_DT", "fp16")

_NC_CACHE = {}


def _build_pos(dlnf):
    """lo/frac per chirp, replicating the reference's fp32 op chain bit-exactly
    (jax-on-CPU); falls back to numpy fp32 if jax is unavailable."""
    try:
        import jax
        import jax.numpy as jnp

        with jax.default_device(jax.devices("cpu")[0]):
            betas = 2.0 * jnp.asarray(dlnf, dtype=jnp.float32)
            tau = jnp.linspace(0.0, 1.0, K)
            safe = jnp.abs(betas) < 1e-8
            betas_safe = jnp.where(safe, jnp.float32(1e-8), betas)
            eb = jnp.exp(betas_safe)
            t_source = 2.0 / betas_safe[:, None] * jnp.log1p(
                tau[None, :] * (eb[:, None] - 1.0)
            ) - 1.0
            identity = jnp.linspace(-1.0, 1.0, K)
            t_source = jnp.where(safe[:, None], identity[None, :], t_source)
            pos = np.asarray((t_source + 1.0) * 0.5 * (K - 1), dtype=np.float32)
            win = np.asarray(
                0.5 * (1.0 - jnp.cos(2.0 * jnp.pi * jnp.arange(K, dtype=jnp.float32) / K)),
                dtype=np.float32,
            )
    except Exception:
        d32 = np.asarray(dlnf, dtype=np.float32)
        betas = (np.float32(2.0) * d32).astype(np.float32)
        tau = np.linspace(0.0, 1.0, K, dtype=np.float32)
        safe = np.abs(betas) < np.float32(1e-8)
        betas_safe = np.where(safe, np.float32(1e-8), betas).astype(np.float32)
        eb = np.exp(betas_safe).astype(np.float32)
        t_source = (np.float32(2.0) / betas_safe)[:, None] * np.log1p(
            tau[None, :] * (eb[:, None] - np.float32(1.0))
        ).astype(np.float32) - np.float32(1.0)
        identity = np.linspace(-1.0, 1.0, K, dtype=np.float32)
        t_source = np.where(safe[:, None], identity[None, :], t_source).astype(np.float32)
        pos = ((t_source + np.float32(1.0)) * np.float32(0.5) * np.float32(K - 1)).astype(np.float32)
        n = np.arange(K, dtype=np.float32)
        win = (np.float32(0.5) * (np.float32(1.0) - np.cos(np.float32(2.0 * np.pi) * n / np.float32(K)))).astype(np.float32)

    lo = np.clip(pos.astype(np.int32), 0, K - 2)
    frac = (pos - lo.astype(np.float32)).astype(np.float32)
    return lo, frac, win


def _build_mats(dlnf):
    """(D, K, OCOLS) float32 combined interp+rDFT matrices, packed-rfft cols."""
    lo, frac, win = _build_pos(dlnf)
    n = np.arange(K, dtype=np.float64)
    f = np.arange(NF, dtype=np.float64)
    E = np.exp(-2j * np.pi * np.outer(n, f) / K)  # (K, NF) c128
    mats = np.empty((D, K, OCOLS), np.float32)
    for d in range(D):
        C = np.zeros((K, NF), np.complex128)
        np.add.at(C, lo[d], E * (1.0 - frac[d].astype(np.float64))[:, None])
        np.add.at(C, lo[d] + 1, E * frac[d].astype(np.float64)[:, None])
        mats[d, :, 0] = C.real[:, 0].astype(np.float32)
        mats[d, :, 1:-1:2] = C.real[:, 1:-1].astype(np.float32)
        mats[d, :, 2:-1:2] = C.imag[:, 1:-1].astype(np.float32)
        mats[d, :, -1] = C.real[:, -1].astype(np.float32)
    return mats, win


def _dtypes(dt_key):
    import concourse.mybir as mybir

    return {
        "bf16": (mybir.dt.bfloat16, mybir.dt.bfloat16),
        "f32": (mybir.dt.float32, mybir.dt.float32),
        "f32r": (mybir.dt.float32r, mybir.dt.float32r),
        "fp16": (mybir.dt.float16, mybir.dt.float16),
    }[dt_key]


def _build_nc(dt_key):
    import concourse.mybir as mybir
    from concourse import bacc
    from concourse.tile import TileContext

    dt_w, dt_m = _dtypes(dt_key)

    nc = bacc.Bacc(
        "TRN2", target_bir_lowering=False, debug=False, num_devices=NCORES
    )
    # partition-major layouts: dram[p, k, :] belongs to SBUF partition p
    wT = nc.declare_dram_parameter("wT", [128, KT, RWS], dt_w, isOutput=False)
    mats = nc.declare_dram_parameter("mats", [DPC, 128, KT, OCOLS], dt_m, isOutput=False)
    # output partition-major: out[c, p, t, :] = result row t*128+p of chirp c
    out = nc.declare_dram_parameter("out", [DPC, 128, NRT, OCOLS], mybir.dt.float32, isOutput=True)

    st_bufs = 4 if dt_key in ("bf16", "fp16") else 2

    with TileContext(nc) as tc:
        with (
            tc.tile_pool(name="wpool", bufs=1) as wpool,
            tc.tile_pool(name="mpool", bufs=1) as mpool,
            tc.tile_pool(name="opool", bufs=st_bufs) as opool,
            tc.tile_pool(name="pspool", bufs=4, space="PSUM") as pspool,
        ):
            # warm up the PE clock (HAM) with throwaway matmuls while the
            # input DMAs are in flight, so real matmuls start at 2.4 GHz
            warm = wpool.tile([128, 512], dt_w, tag="warm", name="warm")
            nc.vector.memset(warm[:], 0.0)
            wps = pspool.tile([128, NCT, CT], mybir.dt.float32, tag="ps", name="warmps")
            for i in range(16):
                nc.tensor.matmul(
                    wps[:, 0, 0:512], warm[:, 0:128], warm[:, 0:512],
                    start=True, stop=True,
                )

            # inputs, split into k-quarters in lockstep across the two HWDGE
            # rings: the pair (wT[2s:2s+2], mats0[2s:2s+2]) lands every ~4.4us,
            # and the PE runs partial-K accumulation stages right behind the
            # arrivals (see prework below).
            wt = wpool.tile([128, KT, RWS], dt_w, tag="w", name="wt")
            m0 = mpool.tile([128, KT, OCOLS], dt_m, tag="m0", name="m0")
            for q in range(4):
                nc.sync.dma_start(out=wt[:, 2 * q:2 * q + 2, :], in_=wT[:, 2 * q:2 * q + 2, :])
                nc.scalar.dma_start(out=m0[:, 2 * q:2 * q + 2, :], in_=mats[0][:, 2 * q:2 * q + 2, :])
            mt = [m0]
            if DPC > 1:
                m1 = mpool.tile([128, KT, OCOLS], dt_m, tag="m1", name="m1")
                nc.scalar.dma_start(out=m1[:], in_=mats[1])
                mt.append(m1)

            def mms(ps, c, r, ks, ke):
                for k in range(ks, ke):
                    for ct in range(NCT):
                        nc.tensor.matmul(
                            ps[:, ct, 0:CT],
                            wt[:, k, r * 128:(r + 1) * 128],
                            mt[c][:, k, ct * CT:(ct + 1) * CT],
                            start=(k == 0),
                            stop=(k == KT - 1),
                        )

            # prework: the first 4 row-tiles of chirp 0 accumulate k-pair
            # stages chasing the quarter-DMA arrivals
            PRE = 4
            st0 = opool.tile([128, GRP, OCOLS], mybir.dt.float32, tag="st", name="st0_0")
            pre_ps = [
                pspool.tile([128, NCT, CT], mybir.dt.float32, tag="ps", name=f"ps0_{rr}")
                for rr in range(PRE)
            ]
            for stg in range(3):
                for rr in range(PRE):
                    mms(pre_ps[rr], 0, rr, 2 * stg, 2 * stg + 2)

            for c in range(DPC):
                groups = [GRP] * NG
                if c == DPC - 1:
                    groups = [GRP] * (NG - 1) + [1] * GRP
                r0 = 0
                for g, gsz in enumerate(groups):
                    if c == 0 and g == 0:
                        st = st0
                    else:
                        st = opool.tile([128, gsz, OCOLS], mybir.dt.float32, tag="st", name=f"st{c}_{g}")
                    last_grp = c == DPC - 1 and g == len(groups) - 1
                    for rr in range(gsz):
                        r = r0 + rr
                        if c == 0 and g == 0 and rr < PRE:
                            ps = pre_ps[rr]
                            mms(ps, 0, r, 6, KT)
                        else:
                            ps = pspool.tile([128, NCT, CT], mybir.dt.float32, tag="ps", name=f"ps{c}_{r}")
                            mms(ps, c, r, 0, KT)
                        if last_grp:
                            # evacuate the final tile per PSUM bank so the
                            # copy+DMA of the first half overlaps the rest
                            for ct in range(NCT):
                                nc.vector.tensor_copy(
                                    out=st[:, rr, ct * CT:(ct + 1) * CT], in_=ps[:, ct, :]
                                )
                                nc.sync.dma_start(
                                    out=out[c][:, r0 + rr:r0 + rr + 1, ct * CT:(ct + 1) * CT],
                                    in_=st[:, rr:rr + 1, ct * CT:(ct + 1) * CT],
                                )
                        else:
                            nc.vector.tensor_copy(out=st[:, rr, :], in_=ps[:].rearrange("p n x -> p (n x)"))
                    if not last_grp:
                        nc.sync.dma_start(out=out[c][:, r0:r0 + gsz, :], in_=st[:, 0:gsz, :])
                    r0 += gsz
    return nc


def _get_nc(dt_key):
    if dt_key not in _NC_CACHE:
        nc = _build_nc(dt_key)
        nc.finalize()
        _NC_CACHE[dt_key] = nc
    return _NC_CACHE[dt_key]


def _cast(arr, half):
    if half == "bf16":
        import ml_dtypes
        return arr.astype(ml_dtypes.bfloat16)
    if half == "fp16":
        return arr.astype(np.float16)
    return arr


def _dev_arrays(x, dlnf, dt_key):
    x = np.asarray(x)
    mats, win = _build_mats(np.asarray(dlnf))
    frames = np.lib.stride_tricks.sliding_window_view(x[0], K)[::HOP]  # (NW, K)
    frames = (frames * win).astype(np.float32)
    wT = np.zeros((K, RWS), np.float32)
    wT[:, :NW] = frames.T
    # partition-major: [128, KT, RWS] with [p, k, :] = wT[k*128+p, :]
    wT_pm = np.ascontiguousarray(wT.reshape(KT, 128, RWS).transpose(1, 0, 2))
    mats_pm = np.ascontiguousarray(
        mats.reshape(D, KT, 128, OCOLS).transpose(0, 2, 1, 3)
    )  # (D, 128, KT, OCOLS)
    half = dt_key if dt_key in ("bf16", "fp16") else None
    return _cast(wT_pm, half), _cast(mats_pm, half)


def kernel(x, dlnf, n_hann_splits):
    assert int(n_hann_splits) == 1
    from concourse.bass_utils import run_bass_kernel_spmd

    dt_key = DEV_DT
    nc = _get_nc(dt_key)
    wT, mats = _dev_arrays(x, dlnf, dt_key)

    core_ids = list(range(NCORES))
    in_maps = [
        {"wT": wT, "mats": np.ascontiguousarray(mats[i * DPC:(i + 1) * DPC])}
        for i in core_ids
    ]
    res = run_bass_kernel_spmd(nc, in_maps, core_ids)

    out = np.empty((D, 1, NW, K), np.complex64)
    outv = out.view(np.float32).reshape(D, 1, NW, K, 2)  # (..., K, 2) re/im
    for i in core_ids:
        dev = res.results[i]["out"]  # (DPC, 128, NRT, OCOLS) f32, partition-major
        rows = np.ascontiguousarray(dev.transpose(0, 2, 1, 3)).reshape(DPC, RWS, OCOLS)[:, :NW, :]
        sl = slice(i * DPC, (i + 1) * DPC)
        outv[sl, 0, :, 0, 0] = rows[:, :, 0]          # re0
        outv[sl, 0, :, 0, 1] = 0.0                    # im0
        outv[sl, 0, :, 1:NF - 1, :] = rows[:, :, 1:-1].reshape(DPC, NW, NF - 2, 2)
        outv[sl, 0, :, NF - 1, 0] = rows[:, :, -1]    # re512
        outv[sl, 0, :, NF - 1, 1] = 0.0               # im512
    out[:, :, :, NF:] = np.conj(out[:, :, :, 1:NF - 1][:, :, :, ::-1])
    return out



# revision 3
# speedup vs baseline: 1.2803x; 1.2803x over previous
"""Dechirp-STFT Trainium2 kernel (mixed fp8/fp16 edition).

Computes, for each of D=16 chirp hypotheses, a resampled (linear-interp)
version of each Hann-windowed signal frame followed by a 1024-point FFT.

Strategy
--------
Per chirp d the whole frame-wise operation (gather/lerp resample -> DFT) is a
single linear map on the 1024-sample frame, so we fold both into one dense
matrix M_d built on the host from `dlnf`:  X_d = frames @ M_d.
Only rFFT bins f=0..512 are computed on device (input frames are real); device
rows use the packed-rfft column order [re0, re1, im1, ..., re511, im511,
re512] (1024 cols; im0/im512 are filled host-side).

Sharding: D axis across the 8 NeuronCores (2 chirps per core).

Precision: the PE runs fp8-e4m3 in DoubleRow mode (2 contraction-tiles per
pass, 2x rate) for the four k-tiles under the Hann window's edges
({0,1,6,7}: ~8% of the window energy, so fp8 quantization there costs only
~1e-2 relative error) and fp16 for the four center k-tiles {2,3,4,5}.
PE stream time: 2 chirps x 16 rowtiles x 2 coltiles x (2 DR + 4 fp16)
instructions = 0.75x the all-fp16 stream. Outputs are written as fp16
(values O(100), gate is 2e-2 — fp16 adds ~1e-4).

Dataflow: inputs are split across the Sync and Scalar HWDGE rings in the
exact order the PE consumes them (fp8 chirp-0 pieces first); the first four
row-tiles of chirp 0 are held as 8 one-bank PSUM accumulation groups whose
six stages chase the DMA arrivals stage-major, so the PE starts ~9us in and
never drains. Remaining row-tiles run group-major at full speed. A short
garbage-warmup burst on the Tensor engine un-throttles the PE clock before
the first real matmul. Output staged GRP=4 row-tiles in fp16 and written on
the Vector ring; the very last 4 row-tiles go per-(row,coltile) to shrink
the tail.
"""

import os
import numpy as np

K = 1024
HOP = 512
NW = 2047          # (1048576 - 1024) // 512 + 1
RWS = 2048         # padded row count (16 x 128 tiles)
NRT = RWS // 128   # 16 row tiles
GRP = 4            # row tiles per output staging group
NG = NRT // GRP
D = 16
NCORES = 8
DPC = D // NCORES  # chirps per core
NF = K // 2 + 1    # 513 rfft bins
OCOLS = 1024
CT = 512           # matmul col-tile (2 x 512 = 1024)
NCT = OCOLS // CT
KT = K // 128      # 8 contraction tiles
FP8_TILES = (0, 1, 6, 7)   # DoubleRow pairs (0,1) and (6,7)
FP16_TILES = (2, 3, 4, 5)
NWARM = 10

DEV_DT = os.environ.get("BASS_KERNEL_DT", "mixed")

_NC_CACHE = {}


def _build_pos(dlnf):
    """lo/frac per chirp, replicating the reference's fp32 op chain bit-exactly
    (jax-on-CPU); falls back to numpy fp32 if jax is unavailable."""
    try:
        import jax
        import jax.numpy as jnp

        with jax.default_device(jax.devices("cpu")[0]):
            betas = 2.0 * jnp.asarray(dlnf, dtype=jnp.float32)
            tau = jnp.linspace(0.0, 1.0, K)
            safe = jnp.abs(betas) < 1e-8
            betas_safe = jnp.where(safe, jnp.float32(1e-8), betas)
            eb = jnp.exp(betas_safe)
            t_source = 2.0 / betas_safe[:, None] * jnp.log1p(
                tau[None, :] * (eb[:, None] - 1.0)
            ) - 1.0
            identity = jnp.linspace(-1.0, 1.0, K)
            t_source = jnp.where(safe[:, None], identity[None, :], t_source)
            pos = np.asarray((t_source + 1.0) * 0.5 * (K - 1), dtype=np.float32)
            win = np.asarray(
                0.5 * (1.0 - jnp.cos(2.0 * jnp.pi * jnp.arange(K, dtype=jnp.float32) / K)),
                dtype=np.float32,
            )
    except Exception:
        d32 = np.asarray(dlnf, dtype=np.float32)
        betas = (np.float32(2.0) * d32).astype(np.float32)
        tau = np.linspace(0.0, 1.0, K, dtype=np.float32)
        safe = np.abs(betas) < np.float32(1e-8)
        betas_safe = np.where(safe, np.float32(1e-8), betas).astype(np.float32)
        eb = np.exp(betas_safe).astype(np.float32)
        t_source = (np.float32(2.0) / betas_safe)[:, None] * np.log1p(
            tau[None, :] * (eb[:, None] - np.float32(1.0))
        ).astype(np.float32) - np.float32(1.0)
        identity = np.linspace(-1.0, 1.0, K, dtype=np.float32)
        t_source = np.where(safe[:, None], identity[None, :], t_source).astype(np.float32)
        pos = ((t_source + np.float32(1.0)) * np.float32(0.5) * np.float32(K - 1)).astype(np.float32)
        n = np.arange(K, dtype=np.float32)
        win = (np.float32(0.5) * (np.float32(1.0) - np.cos(np.float32(2.0 * np.pi) * n / np.float32(K)))).astype(np.float32)

    lo = np.clip(pos.astype(np.int32), 0, K - 2)
    frac = (pos - lo.astype(np.float32)).astype(np.float32)
    return lo, frac, win


def _build_mats(dlnf):
    """(D, K, OCOLS) float32 combined interp+rDFT matrices, packed-rfft cols."""
    lo, frac, win = _build_pos(dlnf)
    n = np.arange(K, dtype=np.float64)
    f = np.arange(NF, dtype=np.float64)
    E = np.exp(-2j * np.pi * np.outer(n, f) / K)  # (K, NF) c128
    mats = np.empty((D, K, OCOLS), np.float32)
    for d in range(D):
        C = np.zeros((K, NF), np.complex128)
        np.add.at(C, lo[d], E * (1.0 - frac[d].astype(np.float64))[:, None])
        np.add.at(C, lo[d] + 1, E * frac[d].astype(np.float64)[:, None])
        mats[d, :, 0] = C.real[:, 0].astype(np.float32)
        mats[d, :, 1:-1:2] = C.real[:, 1:-1].astype(np.float32)
        mats[d, :, 2:-1:2] = C.imag[:, 1:-1].astype(np.float32)
        mats[d, :, -1] = C.real[:, -1].astype(np.float32)
    return mats, win


def _build_nc(dt_key):
    import concourse.mybir as mybir
    from concourse import bacc
    from concourse.tile import TileContext

    DR = mybir.MatmulPerfMode.DoubleRow
    f8 = mybir.dt.float8e4
    f16 = mybir.dt.float16
    f32 = mybir.dt.float32

    nc = bacc.Bacc(
        "TRN2", target_bir_lowering=False, debug=False, num_devices=NCORES
    )
    # partition-major layouts: dram[p, k, :] belongs to SBUF partition p.
    # wT8/m8 hold k-tiles (0,1,6,7) at indices (0,1,2,3); wT16/m16 hold
    # k-tiles (2,3,4,5) at indices (0,1,2,3).
    wT8 = nc.declare_dram_parameter("wT8", [128, 4, RWS], f8, isOutput=False)
    wT16 = nc.declare_dram_parameter("wT16", [128, 4, RWS], f16, isOutput=False)
    m8 = nc.declare_dram_parameter("m8", [DPC, 128, 4, OCOLS], f8, isOutput=False)
    m16 = nc.declare_dram_parameter("m16", [DPC, 128, 4, OCOLS], f16, isOutput=False)
    # output partition-major: out[c, p, t, :] = result row t*128+p of chirp c
    out = nc.declare_dram_parameter("out", [DPC, 128, NRT, OCOLS], f16, isOutput=True)

    with TileContext(nc) as tc:
        with (
            tc.tile_pool(name="wpool", bufs=1) as wpool,
            tc.tile_pool(name="mpool", bufs=1) as mpool,
            tc.tile_pool(name="opool", bufs=4) as opool,
            tc.tile_pool(name="pspool", bufs=8, space="PSUM") as pspool,
        ):
            # warm up the PE clock (HAM) while the input DMAs are in flight,
            # so the first real matmuls run at 2.4 GHz
            warm = wpool.tile([128, 512], f16, tag="warm", name="warm")
            nc.gpsimd.memset(warm[:], 0.0)
            wps = pspool.tile([128, CT], f32, tag="ps", name="warmps")
            for _ in range(NWARM):
                nc.tensor.matmul(
                    wps[:], warm[:, 0:128], warm[:, 0:512], start=True, stop=True
                )

            wt8 = wpool.tile([128, 4, RWS], f8, tag="w8", name="wt8")
            wt16 = wpool.tile([128, 4, RWS], f16, tag="w16", name="wt16")
            m8t = [
                mpool.tile([128, 4, OCOLS], f8, tag=f"m8_{c}", name=f"m8_{c}")
                for c in range(DPC)
            ]
            m16t = [
                mpool.tile([128, 4, OCOLS], f16, tag=f"m16_{c}", name=f"m16_{c}")
                for c in range(DPC)
            ]

            # ---- input DMA program, in PE-consumption order ----
            # sync ring: chirp-0 fp8 pieces, then the sync half of the fp16/
            # column pieces; scalar ring: the other half.
            nc.sync.dma_start(out=m8t[0][:], in_=m8[0])
            nc.sync.dma_start(out=wt8[:, :, 0:512], in_=wT8[:, :, 0:512])
            nc.sync.dma_start(out=m16t[0][:, 0, :], in_=m16[0][:, 0, :])   # k2
            nc.sync.dma_start(out=m16t[0][:, 2, :], in_=m16[0][:, 2, :])   # k4
            nc.sync.dma_start(out=wt16[:, :, 512:1024], in_=wT16[:, :, 512:1024])
            nc.sync.dma_start(out=wt8[:, :, 512:1024], in_=wT8[:, :, 512:1024])
            nc.sync.dma_start(out=wt8[:, :, 1024:2048], in_=wT8[:, :, 1024:2048])
            nc.sync.dma_start(out=m8t[1][:], in_=m8[1])
            nc.sync.dma_start(out=m16t[1][:, 1, :], in_=m16[1][:, 1, :])   # k3
            nc.sync.dma_start(out=m16t[1][:, 3, :], in_=m16[1][:, 3, :])   # k5

            nc.scalar.dma_start(out=wt16[:, :, 0:512], in_=wT16[:, :, 0:512])
            nc.scalar.dma_start(out=m16t[0][:, 1, :], in_=m16[0][:, 1, :])  # k3
            nc.scalar.dma_start(out=m16t[0][:, 3, :], in_=m16[0][:, 3, :])  # k5
            nc.scalar.dma_start(out=wt16[:, :, 1024:2048], in_=wT16[:, :, 1024:2048])
            nc.scalar.dma_start(out=m16t[1][:, 0, :], in_=m16[1][:, 0, :])  # k2
            nc.scalar.dma_start(out=m16t[1][:, 2, :], in_=m16[1][:, 2, :])  # k4

            # one accumulation stage of group (c, r, ct); stages 0/1 are the
            # fp8 DoubleRow pairs, stages 2..5 the fp16 center k-tiles in
            # DMA-arrival order (k2, k4, k3, k5). stop on stage 5.
            def stage_mm(ps, c, r, ct, s):
                rs = slice(r * 128, (r + 1) * 128)
                cs = slice(ct * CT, (ct + 1) * CT)
                if s == 0:
                    nc.tensor.matmul(
                        ps[:], wt8[:, 0:2, rs], m8t[c][:, 0:2, cs],
                        start=True, stop=False, perf_mode=DR,
                    )
                elif s == 1:
                    nc.tensor.matmul(
                        ps[:], wt8[:, 2:4, rs], m8t[c][:, 2:4, cs],
                        start=False, stop=False, perf_mode=DR,
                    )
                else:
                    kt = (0, 2, 1, 3)[s - 2]
                    nc.tensor.matmul(
                        ps[:], wt16[:, kt, rs], m16t[c][:, kt, cs],
                        start=False, stop=(s == 5),
                    )

            # ---- chase: row-tiles 0..3 of chirp 0, stage-major across 8
            # one-bank PSUM groups so the PE tracks the DMA arrivals ----
            pre_ps = {
                (r, ct): pspool.tile([128, CT], f32, tag="ps", name=f"ps0_{r}_{ct}")
                for r in range(GRP) for ct in range(NCT)
            }
            for s in range(6):
                for r in range(GRP):
                    for ct in range(NCT):
                        stage_mm(pre_ps[(r, ct)], 0, r, ct, s)
            st0 = opool.tile([128, GRP, OCOLS], f16, tag="st", name="st0")
            for r in range(GRP):
                for ct in range(NCT):
                    nc.vector.tensor_copy(
                        out=st0[:, r, ct * CT:(ct + 1) * CT], in_=pre_ps[(r, ct)][:]
                    )
            nc.gpsimd.dma_start(out=out[0][:, 0:GRP, :], in_=st0[:, 0:GRP, :])

            # ---- remaining row-tiles, group-major ----
            for c in range(DPC):
                for g in range(NG):
                    if c == 0 and g == 0:
                        continue
                    last_grp = c == DPC - 1 and g == NG - 1
                    if not last_grp:
                        st = opool.tile([128, GRP, OCOLS], f16, tag="st", name=f"st{c}_{g}")
                        for rr in range(GRP):
                            r = g * GRP + rr
                            for ct in range(NCT):
                                ps = pspool.tile([128, CT], f32, tag="ps", name=f"ps{c}_{r}_{ct}")
                                for s in range(6):
                                    stage_mm(ps, c, r, ct, s)
                                nc.vector.tensor_copy(
                                    out=st[:, rr, ct * CT:(ct + 1) * CT], in_=ps[:]
                                )
                        nc.gpsimd.dma_start(
                            out=out[c][:, g * GRP:(g + 1) * GRP, :], in_=st[:, 0:GRP, :]
                        )
                    else:
                        # final 4 row-tiles: per-(row,coltile) copy + DMA so the
                        # tail after the last matmul is as short as possible
                        for rr in range(GRP):
                            r = g * GRP + rr
                            st = opool.tile([128, 1, OCOLS], f16, tag="st", name=f"stL_{rr}")
                            for ct in range(NCT):
                                ps = pspool.tile([128, CT], f32, tag="ps", name=f"ps{c}_{r}_{ct}")
                                for s in range(6):
                                    stage_mm(ps, c, r, ct, s)
                                nc.vector.tensor_copy(
                                    out=st[:, 0, ct * CT:(ct + 1) * CT], in_=ps[:]
                                )
                                nc.gpsimd.dma_start(
                                    out=out[c][:, r:r + 1, ct * CT:(ct + 1) * CT],
                                    in_=st[:, 0:1, ct * CT:(ct + 1) * CT],
                                )
    return nc


def _get_nc(dt_key):
    if dt_key not in _NC_CACHE:
        nc = _build_nc(dt_key)
        nc.finalize()
        _NC_CACHE[dt_key] = nc
    return _NC_CACHE[dt_key]


def _dev_arrays(x, dlnf, dt_key):
    """Returns (wT8, wT16, m8, m16) in device layouts."""
    import ml_dtypes

    f8 = ml_dtypes.float8_e4m3fn
    x = np.asarray(x)
    mats, win = _build_mats(np.asarray(dlnf))
    frames = np.lib.stride_tricks.sliding_window_view(x[0], K)[::HOP]  # (NW, K)
    frames = (frames * win).astype(np.float32)
    wT = np.zeros((K, RWS), np.float32)
    wT[:, :NW] = frames.T
    # partition-major: [128, KT, RWS] with [p, k, :] = wT[k*128+p, :]
    wT_pm = np.ascontiguousarray(wT.reshape(KT, 128, RWS).transpose(1, 0, 2))
    mats_pm = np.ascontiguousarray(
        mats.reshape(D, KT, 128, OCOLS).transpose(0, 2, 1, 3)
    )  # (D, 128, KT, OCOLS)
    wT8 = np.ascontiguousarray(wT_pm[:, FP8_TILES, :]).astype(f8)
    wT16 = np.ascontiguousarray(wT_pm[:, FP16_TILES, :]).astype(np.float16)
    m8 = np.ascontiguousarray(mats_pm[:, :, FP8_TILES, :]).astype(f8)
    m16 = np.ascontiguousarray(mats_pm[:, :, FP16_TILES, :]).astype(np.float16)
    return wT8, wT16, m8, m16


def _in_maps(x, dlnf, dt_key):
    wT8, wT16, m8, m16 = _dev_arrays(x, dlnf, dt_key)
    return [
        {
            "wT8": wT8,
            "wT16": wT16,
            "m8": np.ascontiguousarray(m8[i * DPC:(i + 1) * DPC]),
            "m16": np.ascontiguousarray(m16[i * DPC:(i + 1) * DPC]),
        }
        for i in range(NCORES)
    ]


def kernel(x, dlnf, n_hann_splits):
    assert int(n_hann_splits) == 1
    from concourse.bass_utils import run_bass_kernel_spmd

    dt_key = DEV_DT
    nc = _get_nc(dt_key)
    in_maps = _in_maps(x, dlnf, dt_key)

    core_ids = list(range(NCORES))
    res = run_bass_kernel_spmd(nc, in_maps, core_ids)

    out = np.empty((D, 1, NW, K), np.complex64)
    outv = out.view(np.float32).reshape(D, 1, NW, K, 2)  # (..., K, 2) re/im
    for i in core_ids:
        dev = res.results[i]["out"]  # (DPC, 128, NRT, OCOLS) f16, partition-major
        rows = np.ascontiguousarray(
            dev.transpose(0, 2, 1, 3)
        ).reshape(DPC, RWS, OCOLS)[:, :NW, :].astype(np.float32)
        sl = slice(i * DPC, (i + 1) * DPC)
        outv[sl, 0, :, 0, 0] = rows[:, :, 0]          # re0
        outv[sl, 0, :, 0, 1] = 0.0                    # im0
        outv[sl, 0, :, 1:NF - 1, :] = rows[:, :, 1:-1].reshape(DPC, NW, NF - 2, 2)
        outv[sl, 0, :, NF - 1, 0] = rows[:, :, -1]    # re512
        outv[sl, 0, :, NF - 1, 1] = 0.0               # im512
    out[:, :, :, NF:] = np.conj(out[:, :, :, 1:NF - 1][:, :, :, ::-1])
    return out
